# revision 25
# baseline (speedup 1.0000x reference)
"""Trainium2 Bass kernel for nn_ChebKernelMixture.

Computes gram(xs) = psi(xs) @ psi(xs).T where psi is a Chebyshev feature
map: psi(x) = concat_n sqrt(w_n) * phi_n(x), phi_0 = [1],
phi_n = [T_n(x), sqrt(1-x^2) U_{n-1}(x)], w = softmax(logits).

Shapes: xs (16384,), logits (33,) -> out (16384, 16384) f32.

Strategy (8 NeuronCores, SPMD, no collectives):
  - every core receives the full xs (as xs_all) plus its own 2048-row
    slice (as xs_rows); the program is identical on all cores.
  - Chebyshev features via a half-depth recurrence: degrees 1..16 by the
    usual 3-term chain, degrees 17..32 by the product identity
    X_{16+n} = 2 T_16 X_n - X_{16-n} (X in {T_n, sin n.theta}), whose 15
    op-pairs are mutually independent -- the serial chain latency
    (~450 ns/dependent op on VectorE) only spans 14 steps.
  - features are computed fp32 in slot-reordered range sets sized so the
    first set unlocks the first gemms within ~10 us, converted to fp16
    (VectorE 2x copy), transposed feature-major by TensorE (fp16,
    1 cyc/row) with sqrt(softmax(logits)) folded into the PSUM eviction.
  - each core computes its (2048 x 16384) staircase block of the Gram
    matrix with fp16 TensorE matmuls (K=65) accumulating in fp32 PSUM,
    evicted to fp16 strips split VectorE/ScalarE (the eviction is the
    engine bottleneck) and DMAd out on the SP ring.
  - host concatenates the 8 row blocks, mirrors the lower triangle, and
    upcasts fp16 -> fp32 (tolerance 2e-2; fp16 psi + fp16 out ~1.5e-3).
"""

import sys

if "/opt/trn_rl_repo" not in sys.path:
    sys.path.insert(0, "/opt/trn_rl_repo")

import numpy as np

N_PTS = 16384
MAX_N = 32
N_FEAT = 2 * MAX_N + 1  # 65
N_CORES = 8
ROWS_PER_CORE = N_PTS // N_CORES  # 2048
N_BLOCKS = N_PTS // 128  # 128 column point-blocks
N_ROW_BLOCKS = ROWS_PER_CORE // 128  # 16 row point-blocks
NB = N_BLOCKS + N_ROW_BLOCKS  # 144 XtF slots

# XtF slot -> content ('row', m) or ('gb', gb), ordered so early range
# sets unlock the top of the gemm staircase quickly.
SLOT_CONTENT = (
    [("row", 15)] + [("gb", gb) for gb in range(120, 128)]
    + [("row", 13), ("row", 14)] + [("gb", gb) for gb in range(104, 120)]
    + [("row", m) for m in range(0, 13)]
    + [("gb", gb) for gb in range(96, 104)]
    + [("gb", gb) for gb in range(64, 96)]
    + [("gb", gb) for gb in range(32, 64)]
    + [("gb", gb) for gb in range(0, 32)]
)
assert len(SLOT_CONTENT) == NB
# recurrence range sets over XtF slots; the first tiny set unlocks
# gemm_m(15) within ~10 us
SETS = [(0, 9), (9, 27), (27, 48), (48, 80), (80, 112), (112, 144)]

# of every 12 gemm PSUM tiles, this many evict on VectorE (rest ScalarE)
DVE_EVICT_OF_12 = 4

_CACHE = {}


def _psiA_pos(slot):
    kind, v = SLOT_CONTENT[slot]
    return v * 128 if kind == "row" else ROWS_PER_CORE + v * 128


def _set_of(slot):
    for si, (c0, c1) in enumerate(SETS):
        if c0 <= slot < c1:
            return si, c0, c1
    raise AssertionError(slot)


def _build_nc():
    import concourse.bacc as bacc
    import concourse.tile as tile
    from concourse import mybir
    from concourse.masks import make_identity
    from contextlib import ExitStack

    f32 = mybir.dt.float32
    f16 = mybir.dt.float16
    Act = mybir.ActivationFunctionType
    Alu = mybir.AluOpType

    nc = bacc.Bacc("TRN2", target_bir_lowering=False, debug=False,
                   num_devices=N_CORES)

    xs_all = nc.dram_tensor("xs_all", [128, 128], f32,
                            kind="ExternalInput").ap()
    xs_rows = nc.dram_tensor("xs_rows", [N_ROW_BLOCKS, 128], f32,
                             kind="ExternalInput").ap()
    logits = nc.dram_tensor("logits", [1, MAX_N + 1], f32,
                            kind="ExternalInput").ap()
    # output stored fp16 (tolerance is 2e-2; fp16 adds ~5e-4) -- halves
    # the HBM write traffic, which is the kernel's roofline
    g = nc.dram_tensor("g", [ROWS_PER_CORE, N_PTS], f16,
                       kind="ExternalOutput").ap()

    with tile.TileContext(nc) as tc, ExitStack() as ctx:
        consts = ctx.enter_context(tc.tile_pool(name="consts", bufs=1))
        smalls = ctx.enter_context(tc.tile_pool(name="smalls", bufs=1))
        recp = ctx.enter_context(tc.tile_pool(name="recp", bufs=2))
        tmpp = ctx.enter_context(tc.tile_pool(name="tmpp", bufs=4))
        phip = ctx.enter_context(tc.tile_pool(name="phip", bufs=1))
        psip = ctx.enter_context(tc.tile_pool(name="psip", bufs=1))
        outp = ctx.enter_context(tc.tile_pool(name="outp", bufs=4))
        pre_ps = ctx.enter_context(
            tc.tile_pool(name="pre_ps", bufs=2, space="PSUM"))
        mm_ps = ctx.enter_context(
            tc.tile_pool(name="mm_ps", bufs=3, space="PSUM"))

        # ---- input DMAs -------------------------------------------------
        X = smalls.tile([128, 128], f32, tag="X")
        nc.sync.dma_start(X[:], xs_all[:])
        Xr = smalls.tile([N_ROW_BLOCKS, 128], f32, tag="Xr")
        nc.sync.dma_start(Xr[:], xs_rows[:])
        Lg = smalls.tile([1, MAX_N + 1], f32, tag="Lg")
        nc.sync.dma_start(Lg[:], logits[:])

        # ---- constants --------------------------------------------------
        identity = consts.tile([128, 128], f32, tag="identity")
        make_identity(nc, identity[:])
        id16 = consts.tile([128, 128], f16, tag="id16")
        make_identity(nc, id16[:])
        # dup[j, k] = 1 iff k == 2j or k == 2j-1 (degree-duplication map)
        dup = consts.tile([MAX_N + 1, N_FEAT], f32, tag="dup")
        nc.gpsimd.memset(dup[:], 0.0)
        nc.gpsimd.affine_select(
            out=dup[:], in_=dup[:], compare_op=Alu.not_equal, fill=1.0,
            base=0, pattern=[[-1, N_FEAT]], channel_multiplier=2)
        nc.gpsimd.affine_select(
            out=dup[:], in_=dup[:], compare_op=Alu.not_equal, fill=1.0,
            base=-1, pattern=[[-1, N_FEAT]], channel_multiplier=2)

        # ---- transpose x into slot-ordered point-block layout -----------
        XtF = smalls.tile([128, NB], f32, tag="XtF")
        xtr_ps = mm_ps.tile([128, N_ROW_BLOCKS], f32, tag="ps")
        nc.tensor.transpose(xtr_ps[:], Xr[:],
                            identity[0:N_ROW_BLOCKS, 0:N_ROW_BLOCKS])
        xt_ps = mm_ps.tile([128, 128], f32, tag="ps")
        nc.tensor.transpose(xt_ps[:], X[:], identity[:])
        # copy PSUM transposes into XtF slot order, in contiguous runs
        runs = []
        s = 0
        while s < NB:
            kind, v = SLOT_CONTENT[s]
            e = s + 1
            while e < NB and SLOT_CONTENT[e] == (kind, v + (e - s)):
                e += 1
            runs.append((s, e, kind, v))
            s = e
        for (s, e, kind, v) in runs:
            src = xtr_ps if kind == "row" else xt_ps
            nc.any.tensor_copy(XtF[:, s:e], src[:, v:v + (e - s)])

        # ---- softmax(logits) -> sqrt weights, expanded per feature -----
        SW65 = smalls.tile([N_FEAT, 1], f32, tag="SW65")

        def softmax_weights():
            E = smalls.tile([1, MAX_N + 1], f32, tag="E")
            nc.scalar.activation(E[:], Lg[:], Act.Exp)
            S = smalls.tile([1, 1], f32, tag="S")
            nc.vector.tensor_reduce(S[:], E[:], axis=mybir.AxisListType.X,
                                    op=Alu.add)
            R = smalls.tile([1, 1], f32, tag="R")
            nc.vector.reciprocal(R[:], S[:])
            W = smalls.tile([1, MAX_N + 1], f32, tag="W")
            nc.vector.tensor_scalar_mul(W[:], E[:], R[:])
            SW = smalls.tile([1, MAX_N + 1], f32, tag="SW")
            nc.scalar.activation(SW[:], W[:], Act.Sqrt)
            swc_ps = mm_ps.tile([MAX_N + 1, 1], f32, tag="ps")
            nc.tensor.transpose(swc_ps[:], SW[:], identity[0:1, 0:1])
            SWc = smalls.tile([MAX_N + 1, 1], f32, tag="SWc")
            nc.any.tensor_copy(SWc[:], swc_ps[:])
            sw65_ps = mm_ps.tile([N_FEAT, 1], f32, tag="ps")
            nc.tensor.matmul(sw65_ps[:], dup[:], SWc[:], start=True,
                             stop=True)
            nc.any.tensor_copy(SW65[:], sw65_ps[:])

        # ---- Chebyshev features, half-depth recurrence per range set ----
        # feature order: 0 -> 1;  2k-1 -> T_k;  2k -> S_k = sin(k acos x)
        PHI32 = []
        PHI16 = []
        for si, (c0, c1) in enumerate(SETS):
            cw = c1 - c0
            PHI32.append(phip.tile([128, N_FEAT, cw], f32, tag=f"PHI{si}",
                                   name=f"PHI{si}"))
            PHI16.append(phip.tile([128, N_FEAT, cw], f16,
                                   tag=f"PHI16_{si}", name=f"PHI16_{si}"))
        psiA = psip.tile([N_FEAT, NB * 128], f16, tag="psiA")

        def rec_chain_ops(si):
            """Emit-closures for the serial chain of set si, in dependency
            order; the caller interleaves two sets' chains so their
            ~200 ns inter-op dependency bubbles hide each other."""
            c0, c1 = SETS[si]
            cw = c1 - c0
            P = PHI32[si]
            xc = XtF[:, c0:c1]
            x2 = recp.tile([128, cw], f32, tag=f"x2_{si % 2}",
                           name=f"x2_{si}")
            x2d2 = recp.tile([128, 2, cw], f32, tag=f"x2d2_{si % 2}",
                             name=f"x2d2_{si}")
            ops = []
            ops.append(lambda: nc.vector.tensor_mul(x2[:], xc, xc))
            ops.append(lambda: nc.vector.tensor_scalar_mul(
                x2d2[:, 0, :], xc, 2.0))
            ops.append(lambda: nc.vector.tensor_scalar_mul(
                x2d2[:, 1, :], xc, 2.0))
            ops.append(lambda: nc.vector.memset(P[:, 0, :], 1.0))
            ops.append(lambda: nc.vector.tensor_copy(P[:, 1, :], xc))
            # s = sqrt(1 - x^2) on ScalarE (|x| <= 1)
            ops.append(lambda: nc.scalar.activation(
                P[:, 2, :], x2[:], Act.Sqrt, bias=1.0, scale=-1.0))
            ops.append(lambda: nc.vector.tensor_scalar(
                P[:, 3, :], x2[:], 2.0, -1.0, op0=Alu.mult, op1=Alu.add))
            ops.append(lambda: nc.vector.tensor_mul(
                P[:, 4, :], x2d2[:, 0, :], P[:, 2, :]))
            # chain: (T_n, S_n) = 2x*(T_{n-1}, S_{n-1}) - (T_{n-2}, S_{n-2})
            for n in range(3, 17):
                tmp_holder = {}

                def op_mul(n=n, h=tmp_holder):
                    tmp = tmpp.tile([128, 2, cw], f32, tag="tmp",
                                    name=f"tmp_{si}_{n}")
                    nc.vector.tensor_mul(
                        tmp[:], P[:, 2 * n - 3:2 * n - 1, :], x2d2[:])
                    h["t"] = tmp

                def op_sub(n=n, h=tmp_holder):
                    nc.vector.tensor_sub(
                        P[:, 2 * n - 1:2 * n + 1, :], h["t"][:],
                        P[:, 2 * n - 5:2 * n - 3, :])

                ops.append(op_mul)
                ops.append(op_sub)
            return ops

        def rec_chains(si_a, si_b):
            # 2:1 weighted interleave -- set a (needed first by the
            # staircase) finishes sooner while set b still hides a's
            # dependency bubbles
            oa, ob = rec_chain_ops(si_a), rec_chain_ops(si_b)
            ia = ib = 0
            while ia < len(oa) or ib < len(ob):
                for _ in range(2):
                    if ia < len(oa):
                        oa[ia]()
                        ia += 1
                if ib < len(ob):
                    ob[ib]()
                    ib += 1

        def rec_par(si):
            # doubling: X_{16+n} = 2 T_16 X_n - X_{16-n}; the 15 op-pairs
            # below are mutually independent (no serial chain)
            c0, c1 = SETS[si]
            cw = c1 - c0
            P = PHI32[si]
            D = recp.tile([128, 2, cw], f32, tag=f"D_{si % 2}",
                          name=f"D_{si}")
            nc.vector.tensor_scalar_mul(D[:, 0, :], P[:, 31, :], 2.0)
            nc.vector.tensor_scalar_mul(D[:, 1, :], P[:, 31, :], 2.0)
            for n in range(1, 16):
                k = 16 - n
                tmp = tmpp.tile([128, 2, cw], f32, tag="tmp",
                                name=f"ptmp_{si}_{n}")
                nc.vector.tensor_mul(tmp[:], P[:, 2 * n - 1:2 * n + 1, :],
                                     D[:])
                nc.vector.tensor_sub(
                    P[:, 2 * (16 + n) - 1:2 * (16 + n) + 1, :], tmp[:],
                    P[:, 2 * k - 1:2 * k + 1, :])
            # n = 16: T_32 = 2 T_16 T_16 - 1, S_32 = 2 T_16 S_16 - 0
            t32 = tmpp.tile([128, cw], f32, tag="t32", name=f"t32_{si}")
            nc.vector.tensor_mul(t32[:], D[:, 0, :], P[:, 31, :])
            nc.vector.tensor_scalar(P[:, 63, :], t32[:], -1.0, 1.0,
                                    op0=Alu.add, op1=Alu.mult)
            nc.vector.tensor_mul(P[:, 64, :], D[:, 0, :], P[:, 32, :])

        def convert_set(si):
            # fp32 -> fp16 twin (dense, 2x-eligible copy on VectorE)
            nc.vector.tensor_copy(PHI16[si][:], PHI32[si][:])

        tr_ctr = [0]

        def transposes(s0, s1):
            # psi^T blocks (fp16, 1 cyc/row on PE) with the sqrt(w) row
            # scaling folded into the PSUM->SBUF eviction on ScalarE. Up
            # to 8 transposes share one PSUM tile and one eviction op
            # (psiA destinations contiguous, single range set).
            b = s0
            while b < s1:
                si, cc0, cc1 = _set_of(b)
                g_ = min(8, s1 - b, cc1 - b)
                while g_ > 1 and (_psiA_pos(b + g_ - 1)
                                  != _psiA_pos(b) + (g_ - 1) * 128):
                    g_ -= 1
                tps = pre_ps.tile([N_FEAT, g_ * 128], f16, tag="pre")
                for i in range(g_):
                    nc.tensor.transpose(tps[:, i * 128:(i + 1) * 128],
                                        PHI16[si][:, :, b + i - cc0],
                                        id16[:])
                p0 = _psiA_pos(b)
                # psiA evictions stay on ScalarE: a VectorE eviction here
                # would queue behind in-flight recurrence chains and stall
                # the dependent gemm
                nc.scalar.mul(psiA[:, p0:p0 + g_ * 128], tps[:], SW65[:])
                b += g_

        evict_ctr = [0]

        def gemm_m(m):
            # symmetric staircase: row tile m (global row tile 8m+core)
            # computes Gram cols [1024m, 16384); the host mirrors the
            # rest from G[i,j] = G[j,i] (bit-exact on device).
            lhsT = psiA[:, m * 128:(m + 1) * 128]
            # taper the very last strips so the evict+DMA pipeline drains
            # quickly after the final matmul
            widths = [8192, 4096, 2048, 1024, 1024] if m == 0 else None
            cs = m * 1024
            wi = 0
            while cs < N_PTS:
                w = min(widths[wi] if widths else 8192, N_PTS - cs)
                wi += 1
                strip = outp.tile([128, w], f16, tag="strip")
                for j in range(w // 1024):
                    c = ROWS_PER_CORE + cs + j * 1024
                    ps = mm_ps.tile([128, 1024], f32, tag="ps")
                    nc.tensor.matmul(ps[:, 0:512], lhsT,
                                     psiA[:, c:c + 512],
                                     start=True, stop=True)
                    nc.tensor.matmul(ps[:, 512:1024], lhsT,
                                     psiA[:, c + 512:c + 1024],
                                     start=True, stop=True)
                    # PSUM->SBUF eviction is the engine bottleneck: split
                    # tiles between VectorE and ScalarE
                    t = evict_ctr[0]
                    evict_ctr[0] += 1
                    dst = strip[:, j * 1024:(j + 1) * 1024]
                    if t % 12 < DVE_EVICT_OF_12:
                        nc.vector.tensor_copy(dst, ps[:])
                    else:
                        nc.scalar.mul(dst, ps[:], 1.0)
                # single HWDGE ring (SP): keeps DMA dispatch off ACT,
                # which is loaded with evictions
                nc.sync.dma_start(g[m * 128:(m + 1) * 128, cs:cs + w],
                                  strip[:])
                cs += w

        # slot ranges, by content, for transpose scheduling
        def slots_of_gbs(glo, ghi):
            return [s for s in range(NB)
                    if SLOT_CONTENT[s][0] == "gb"
                    and glo <= SLOT_CONTENT[s][1] < ghi]

        def slots_of_rows(mlo, mhi):
            return [s for s in range(NB)
                    if SLOT_CONTENT[s][0] == "row"
                    and mlo <= SLOT_CONTENT[s][1] < mhi]

        def transpose_slots(slots):
            slots = sorted(slots)
            i = 0
            while i < len(slots):
                j = i + 1
                while j < len(slots) and slots[j] == slots[i] + (j - i):
                    j += 1
                transposes(slots[i], slots[j - 1] + 1)
                i = j

        # ---- pipelined emission, staircase top-down ---------------------
        rec_chains(0, 1)        # set0: row15+gb120..127; set1: r13,14+gb104..119
        softmax_weights()
        rec_par(0)
        convert_set(0)
        transpose_slots(slots_of_rows(15, 16) + slots_of_gbs(120, 128))
        gemm_m(15)
        rec_par(1)
        convert_set(1)
        transpose_slots(slots_of_gbs(112, 120) + slots_of_rows(13, 15))
        gemm_m(14)
        rec_chains(2, 3)        # set2: rows 0..12+gb96..103; set3: gb64..95
        transpose_slots(slots_of_gbs(104, 112))
        gemm_m(13)
        rec_par(2)
        convert_set(2)
        transpose_slots(slots_of_gbs(96, 104) + slots_of_rows(8, 13))
        gemm_m(12)
        rec_par(3)
        convert_set(3)
        transpose_slots(slots_of_gbs(88, 96) + slots_of_rows(4, 8))
        gemm_m(11)
        transpose_slots(slots_of_gbs(80, 88))
        gemm_m(10)
        rec_chains(4, 5)        # set4: gb 32..63; set5: gb 0..31
        transpose_slots(slots_of_gbs(72, 80))
        gemm_m(9)
        transpose_slots(slots_of_gbs(64, 72))
        gemm_m(8)
        rec_par(4)
        convert_set(4)
        transpose_slots(slots_of_gbs(56, 64) + slots_of_rows(0, 4))
        gemm_m(7)
        transpose_slots(slots_of_gbs(48, 56))
        gemm_m(6)
        transpose_slots(slots_of_gbs(40, 48))
        gemm_m(5)
        rec_par(5)
        convert_set(5)
        transpose_slots(slots_of_gbs(32, 40))
        gemm_m(4)
        transpose_slots(slots_of_gbs(24, 32))
        gemm_m(3)
        transpose_slots(slots_of_gbs(16, 24))
        gemm_m(2)
        transpose_slots(slots_of_gbs(8, 16))
        gemm_m(1)
        transpose_slots(slots_of_gbs(0, 8))
        gemm_m(0)

    nc.compile()
    return nc


def _get_nc():
    if "nc" not in _CACHE:
        _CACHE["nc"] = _build_nc()
    return _CACHE["nc"]


def _make_in_maps(xs, logits):
    xs = np.ascontiguousarray(np.asarray(xs, dtype=np.float32).reshape(N_PTS))
    lg = np.ascontiguousarray(
        np.asarray(logits, dtype=np.float32).reshape(1, MAX_N + 1))
    xa = xs.reshape(128, 128)
    in_maps = []
    for c in range(N_CORES):
        # row tile m of core c is global row tile 8m+c
        rows = np.stack([xs[1024 * m + 128 * c:1024 * m + 128 * (c + 1)]
                         for m in range(N_ROW_BLOCKS)])
        in_maps.append({
            "xs_all": xa,
            "xs_rows": np.ascontiguousarray(rows),
            "logits": lg,
        })
    return in_maps


def run(xs, logits, trace=False, tmpdir=None):
    """Run the SPMD kernel; returns (full output, BassKernelResults)."""
    from concourse.bass_utils import run_bass_kernel_spmd

    nc = _get_nc()
    in_maps = _make_in_maps(xs, logits)
    res = run_bass_kernel_spmd(nc, in_maps, list(range(N_CORES)),
                               trace=trace, tmpdir=tmpdir)
    # assemble the upper staircase, then mirror the strict lower
    # triangle (device computes G[i,j] and G[j,i] identically, so the
    # mirror is bit-exact)
    out = np.zeros((N_PTS, N_PTS), np.float32)
    for c in range(N_CORES):
        gc = np.asarray(res.results[c]["g"], dtype=np.float32)
        for m in range(N_ROW_BLOCKS):
            r0 = 1024 * m + 128 * c
            out[r0:r0 + 128, 1024 * m:] = gc[128 * m:128 * (m + 1),
                                             1024 * m:]
    for m in range(1, N_ROW_BLOCKS):
        out[1024 * m:1024 * (m + 1), 0:1024 * m] = \
            out[0:1024 * m, 1024 * m:1024 * (m + 1)].T
    return out, res


def kernel(xs, logits):
    out, _ = run(xs, logits, trace=False)
    return out


# revision 27
# speedup vs baseline: 1.0329x; 1.0329x over previous
"""Trainium2 Bass kernel for nn_ChebKernelMixture.

Computes gram(xs) = psi(xs) @ psi(xs).T where psi is a Chebyshev feature
map: psi(x) = concat_n sqrt(w_n) * phi_n(x), phi_0 = [1],
phi_n = [T_n(x), sqrt(1-x^2) U_{n-1}(x)], w = softmax(logits).

Shapes: xs (16384,), logits (33,) -> out (16384, 16384) f32.

Strategy (8 NeuronCores, SPMD, no collectives):
  - every core receives the full xs (as xs_all) plus its own 2048-row
    slice (as xs_rows); the program is identical on all cores.
  - Chebyshev features via a half-depth recurrence: degrees 1..16 by the
    usual 3-term chain, degrees 17..32 by the product identity
    X_{16+n} = 2 T_16 X_n - X_{16-n} (X in {T_n, sin n.theta}), whose 15
    op-pairs are mutually independent -- the serial chain latency
    (~450 ns/dependent op on VectorE) only spans 14 steps.
  - features are computed fp32 in slot-reordered range sets sized so the
    first set unlocks the first gemms within ~10 us, converted to fp16
    (VectorE 2x copy), transposed feature-major by TensorE (fp16,
    1 cyc/row) with sqrt(softmax(logits)) folded into the PSUM eviction.
  - each core computes its (2048 x 16384) staircase block of the Gram
    matrix with fp16 TensorE matmuls (K=65) accumulating in fp32 PSUM,
    evicted to fp16 strips split VectorE/ScalarE (the eviction is the
    engine bottleneck) and DMAd out on the SP ring.
  - host concatenates the 8 row blocks, mirrors the lower triangle, and
    upcasts fp16 -> fp32 (tolerance 2e-2; fp16 psi + fp16 out ~1.5e-3).
"""

import sys

if "/opt/trn_rl_repo" not in sys.path:
    sys.path.insert(0, "/opt/trn_rl_repo")

import numpy as np

N_PTS = 16384
MAX_N = 32
N_FEAT = 2 * MAX_N + 1  # 65
N_CORES = 8
ROWS_PER_CORE = N_PTS // N_CORES  # 2048
N_BLOCKS = N_PTS // 128  # 128 column point-blocks
N_ROW_BLOCKS = ROWS_PER_CORE // 128  # 16 row point-blocks
NB = N_BLOCKS + N_ROW_BLOCKS  # 144 XtF slots

# XtF slot -> content ('row', m) or ('gb', gb), ordered so early range
# sets unlock the top of the gemm staircase quickly.
SLOT_CONTENT = (
    [("row", 15)] + [("gb", gb) for gb in range(120, 128)]
    + [("row", 13), ("row", 14)] + [("gb", gb) for gb in range(104, 120)]
    + [("row", m) for m in range(0, 13)]
    + [("gb", gb) for gb in range(96, 104)]
    + [("gb", gb) for gb in range(64, 96)]
    + [("gb", gb) for gb in range(32, 64)]
    + [("gb", gb) for gb in range(0, 32)]
)
assert len(SLOT_CONTENT) == NB
# recurrence range sets over XtF slots; the first tiny set unlocks
# gemm_m(15) within ~10 us
SETS = [(0, 9), (9, 27), (27, 48), (48, 80), (80, 112), (112, 144)]

# of every 12 gemm PSUM tiles, this many evict on VectorE (rest ScalarE)
DVE_EVICT_OF_12 = 3

_CACHE = {}


def _psiA_pos(slot):
    kind, v = SLOT_CONTENT[slot]
    return v * 128 if kind == "row" else ROWS_PER_CORE + v * 128


def _set_of(slot):
    for si, (c0, c1) in enumerate(SETS):
        if c0 <= slot < c1:
            return si, c0, c1
    raise AssertionError(slot)


def _build_nc():
    import concourse.bacc as bacc
    import concourse.tile as tile
    from concourse import mybir
    from concourse.masks import make_identity
    from contextlib import ExitStack

    f32 = mybir.dt.float32
    f16 = mybir.dt.float16
    Act = mybir.ActivationFunctionType
    Alu = mybir.AluOpType

    nc = bacc.Bacc("TRN2", target_bir_lowering=False, debug=False,
                   num_devices=N_CORES)

    xs_all = nc.dram_tensor("xs_all", [128, 128], f32,
                            kind="ExternalInput").ap()
    xs_rows = nc.dram_tensor("xs_rows", [N_ROW_BLOCKS, 128], f32,
                             kind="ExternalInput").ap()
    logits = nc.dram_tensor("logits", [1, MAX_N + 1], f32,
                            kind="ExternalInput").ap()
    # output stored fp16 (tolerance is 2e-2; fp16 adds ~5e-4) -- halves
    # the HBM write traffic, which is the kernel's roofline
    g = nc.dram_tensor("g", [ROWS_PER_CORE, N_PTS], f16,
                       kind="ExternalOutput").ap()

    with tile.TileContext(nc) as tc, ExitStack() as ctx:
        consts = ctx.enter_context(tc.tile_pool(name="consts", bufs=1))
        smalls = ctx.enter_context(tc.tile_pool(name="smalls", bufs=1))
        recp = ctx.enter_context(tc.tile_pool(name="recp", bufs=2))
        tmpp = ctx.enter_context(tc.tile_pool(name="tmpp", bufs=4))
        phip = ctx.enter_context(tc.tile_pool(name="phip", bufs=1))
        psip = ctx.enter_context(tc.tile_pool(name="psip", bufs=1))
        outp = ctx.enter_context(tc.tile_pool(name="outp", bufs=4))
        pre_ps = ctx.enter_context(
            tc.tile_pool(name="pre_ps", bufs=2, space="PSUM"))
        mm_ps = ctx.enter_context(
            tc.tile_pool(name="mm_ps", bufs=3, space="PSUM"))

        # ---- input DMAs -------------------------------------------------
        X = smalls.tile([128, 128], f32, tag="X")
        nc.sync.dma_start(X[:], xs_all[:])
        Xr = smalls.tile([N_ROW_BLOCKS, 128], f32, tag="Xr")
        nc.sync.dma_start(Xr[:], xs_rows[:])
        Lg = smalls.tile([1, MAX_N + 1], f32, tag="Lg")
        nc.sync.dma_start(Lg[:], logits[:])

        # ---- constants --------------------------------------------------
        identity = consts.tile([128, 128], f32, tag="identity")
        make_identity(nc, identity[:])
        id16 = consts.tile([128, 128], f16, tag="id16")
        make_identity(nc, id16[:])
        # dup[j, k] = 1 iff k == 2j or k == 2j-1 (degree-duplication map)
        dup = consts.tile([MAX_N + 1, N_FEAT], f32, tag="dup")
        nc.gpsimd.memset(dup[:], 0.0)
        nc.gpsimd.affine_select(
            out=dup[:], in_=dup[:], compare_op=Alu.not_equal, fill=1.0,
            base=0, pattern=[[-1, N_FEAT]], channel_multiplier=2)
        nc.gpsimd.affine_select(
            out=dup[:], in_=dup[:], compare_op=Alu.not_equal, fill=1.0,
            base=-1, pattern=[[-1, N_FEAT]], channel_multiplier=2)

        # ---- transpose x into slot-ordered point-block layout -----------
        XtF = smalls.tile([128, NB], f32, tag="XtF")
        xtr_ps = mm_ps.tile([128, N_ROW_BLOCKS], f32, tag="ps")
        nc.tensor.transpose(xtr_ps[:], Xr[:],
                            identity[0:N_ROW_BLOCKS, 0:N_ROW_BLOCKS])
        xt_ps = mm_ps.tile([128, 128], f32, tag="ps")
        nc.tensor.transpose(xt_ps[:], X[:], identity[:])
        # copy PSUM transposes into XtF slot order, in contiguous runs
        runs = []
        s = 0
        while s < NB:
            kind, v = SLOT_CONTENT[s]
            e = s + 1
            while e < NB and SLOT_CONTENT[e] == (kind, v + (e - s)):
                e += 1
            runs.append((s, e, kind, v))
            s = e
        for (s, e, kind, v) in runs:
            src = xtr_ps if kind == "row" else xt_ps
            nc.any.tensor_copy(XtF[:, s:e], src[:, v:v + (e - s)])

        # ---- softmax(logits) -> sqrt weights, expanded per feature -----
        SW65 = smalls.tile([N_FEAT, 1], f32, tag="SW65")

        def softmax_weights():
            E = smalls.tile([1, MAX_N + 1], f32, tag="E")
            nc.scalar.activation(E[:], Lg[:], Act.Exp)
            S = smalls.tile([1, 1], f32, tag="S")
            nc.vector.tensor_reduce(S[:], E[:], axis=mybir.AxisListType.X,
                                    op=Alu.add)
            R = smalls.tile([1, 1], f32, tag="R")
            nc.vector.reciprocal(R[:], S[:])
            W = smalls.tile([1, MAX_N + 1], f32, tag="W")
            nc.vector.tensor_scalar_mul(W[:], E[:], R[:])
            SW = smalls.tile([1, MAX_N + 1], f32, tag="SW")
            nc.scalar.activation(SW[:], W[:], Act.Sqrt)
            swc_ps = mm_ps.tile([MAX_N + 1, 1], f32, tag="ps")
            nc.tensor.transpose(swc_ps[:], SW[:], identity[0:1, 0:1])
            SWc = smalls.tile([MAX_N + 1, 1], f32, tag="SWc")
            nc.any.tensor_copy(SWc[:], swc_ps[:])
            sw65_ps = mm_ps.tile([N_FEAT, 1], f32, tag="ps")
            nc.tensor.matmul(sw65_ps[:], dup[:], SWc[:], start=True,
                             stop=True)
            nc.any.tensor_copy(SW65[:], sw65_ps[:])

        # ---- Chebyshev features, half-depth recurrence per range set ----
        # feature order: 0 -> 1;  2k-1 -> T_k;  2k -> S_k = sin(k acos x)
        PHI32 = []
        PHI16 = []
        for si, (c0, c1) in enumerate(SETS):
            cw = c1 - c0
            PHI32.append(phip.tile([128, N_FEAT, cw], f32, tag=f"PHI{si}",
                                   name=f"PHI{si}"))
            PHI16.append(phip.tile([128, N_FEAT, cw], f16,
                                   tag=f"PHI16_{si}", name=f"PHI16_{si}"))
        psiA = psip.tile([N_FEAT, NB * 128], f16, tag="psiA")

        def rec_chain_ops(si):
            """Emit-closures for the serial chain of set si, in dependency
            order; the caller interleaves two sets' chains so their
            ~200 ns inter-op dependency bubbles hide each other."""
            c0, c1 = SETS[si]
            cw = c1 - c0
            P = PHI32[si]
            xc = XtF[:, c0:c1]
            x2 = recp.tile([128, cw], f32, tag=f"x2_{si % 2}",
                           name=f"x2_{si}")
            x2d2 = recp.tile([128, 2, cw], f32, tag=f"x2d2_{si % 2}",
                             name=f"x2d2_{si}")
            ops = []
            ops.append(lambda: nc.vector.tensor_mul(x2[:], xc, xc))
            ops.append(lambda: nc.vector.tensor_scalar_mul(
                x2d2[:, 0, :], xc, 2.0))
            ops.append(lambda: nc.vector.tensor_scalar_mul(
                x2d2[:, 1, :], xc, 2.0))
            ops.append(lambda: nc.vector.memset(P[:, 0, :], 1.0))
            ops.append(lambda: nc.vector.tensor_copy(P[:, 1, :], xc))
            # s = sqrt(1 - x^2) on ScalarE (|x| <= 1)
            ops.append(lambda: nc.scalar.activation(
                P[:, 2, :], x2[:], Act.Sqrt, bias=1.0, scale=-1.0))
            ops.append(lambda: nc.vector.tensor_scalar(
                P[:, 3, :], x2[:], 2.0, -1.0, op0=Alu.mult, op1=Alu.add))
            ops.append(lambda: nc.vector.tensor_mul(
                P[:, 4, :], x2d2[:, 0, :], P[:, 2, :]))
            # chain: (T_n, S_n) = 2x*(T_{n-1}, S_{n-1}) - (T_{n-2}, S_{n-2})
            for n in range(3, 17):
                tmp_holder = {}

                def op_mul(n=n, h=tmp_holder):
                    tmp = tmpp.tile([128, 2, cw], f32, tag="tmp",
                                    name=f"tmp_{si}_{n}")
                    nc.vector.tensor_mul(
                        tmp[:], P[:, 2 * n - 3:2 * n - 1, :], x2d2[:])
                    h["t"] = tmp

                def op_sub(n=n, h=tmp_holder):
                    nc.vector.tensor_sub(
                        P[:, 2 * n - 1:2 * n + 1, :], h["t"][:],
                        P[:, 2 * n - 5:2 * n - 3, :])

                ops.append(op_mul)
                ops.append(op_sub)
            return ops

        def rec_chains(si_a, si_b):
            # 1:1 interleave -- each set's dependency bubble is hidden by
            # the other set's op
            oa, ob = rec_chain_ops(si_a), rec_chain_ops(si_b)
            for i in range(max(len(oa), len(ob))):
                if i < len(oa):
                    oa[i]()
                if i < len(ob):
                    ob[i]()

        def rec_par(si):
            # doubling: X_{16+n} = 2 T_16 X_n - X_{16-n}; the 15 op-pairs
            # below are mutually independent (no serial chain)
            c0, c1 = SETS[si]
            cw = c1 - c0
            P = PHI32[si]
            D = recp.tile([128, 2, cw], f32, tag=f"D_{si % 2}",
                          name=f"D_{si}")
            nc.vector.tensor_scalar_mul(D[:, 0, :], P[:, 31, :], 2.0)
            nc.vector.tensor_scalar_mul(D[:, 1, :], P[:, 31, :], 2.0)
            for n in range(1, 16):
                k = 16 - n
                tmp = tmpp.tile([128, 2, cw], f32, tag="tmp",
                                name=f"ptmp_{si}_{n}")
                nc.vector.tensor_mul(tmp[:], P[:, 2 * n - 1:2 * n + 1, :],
                                     D[:])
                nc.vector.tensor_sub(
                    P[:, 2 * (16 + n) - 1:2 * (16 + n) + 1, :], tmp[:],
                    P[:, 2 * k - 1:2 * k + 1, :])
            # n = 16: T_32 = 2 T_16 T_16 - 1, S_32 = 2 T_16 S_16 - 0
            t32 = tmpp.tile([128, cw], f32, tag="t32", name=f"t32_{si}")
            nc.vector.tensor_mul(t32[:], D[:, 0, :], P[:, 31, :])
            nc.vector.tensor_scalar(P[:, 63, :], t32[:], -1.0, 1.0,
                                    op0=Alu.add, op1=Alu.mult)
            nc.vector.tensor_mul(P[:, 64, :], D[:, 0, :], P[:, 32, :])

        def convert_set(si):
            # fp32 -> fp16 twin (dense, 2x-eligible copy on VectorE)
            nc.vector.tensor_copy(PHI16[si][:], PHI32[si][:])

        tr_ctr = [0]

        def transposes(s0, s1):
            # psi^T blocks (fp16, 1 cyc/row on PE) with the sqrt(w) row
            # scaling folded into the PSUM->SBUF eviction on ScalarE. Up
            # to 8 transposes share one PSUM tile and one eviction op
            # (psiA destinations contiguous, single range set).
            b = s0
            while b < s1:
                si, cc0, cc1 = _set_of(b)
                g_ = min(8, s1 - b, cc1 - b)
                while g_ > 1 and (_psiA_pos(b + g_ - 1)
                                  != _psiA_pos(b) + (g_ - 1) * 128):
                    g_ -= 1
                tps = pre_ps.tile([N_FEAT, g_ * 128], f16, tag="pre")
                for i in range(g_):
                    nc.tensor.transpose(tps[:, i * 128:(i + 1) * 128],
                                        PHI16[si][:, :, b + i - cc0],
                                        id16[:])
                p0 = _psiA_pos(b)
                # psiA evictions stay on ScalarE: a VectorE eviction here
                # would queue behind in-flight recurrence chains and stall
                # the dependent gemm
                nc.scalar.mul(psiA[:, p0:p0 + g_ * 128], tps[:], SW65[:])
                b += g_

        evict_ctr = [0]

        def gemm_m(m):
            # symmetric staircase: row tile m (global row tile 8m+core)
            # computes Gram cols [1024m, 16384); the host mirrors the
            # rest from G[i,j] = G[j,i] (bit-exact on device).
            lhsT = psiA[:, m * 128:(m + 1) * 128]
            # taper the very last strips so the evict+DMA pipeline drains
            # quickly after the final matmul
            widths = [8192, 4096, 2048, 1024, 1024] if m == 0 else None
            cs = m * 1024
            wi = 0
            while cs < N_PTS:
                w = min(widths[wi] if widths else 8192, N_PTS - cs)
                wi += 1
                strip = outp.tile([128, w], f16, tag="strip")
                for j in range(w // 1024):
                    c = ROWS_PER_CORE + cs + j * 1024
                    ps = mm_ps.tile([128, 1024], f32, tag="ps")
                    nc.tensor.matmul(ps[:, 0:512], lhsT,
                                     psiA[:, c:c + 512],
                                     start=True, stop=True)
                    nc.tensor.matmul(ps[:, 512:1024], lhsT,
                                     psiA[:, c + 512:c + 1024],
                                     start=True, stop=True)
                    # PSUM->SBUF eviction is the engine bottleneck: split
                    # tiles between VectorE and ScalarE
                    t = evict_ctr[0]
                    evict_ctr[0] += 1
                    dst = strip[:, j * 1024:(j + 1) * 1024]
                    if t % 12 < DVE_EVICT_OF_12:
                        nc.vector.tensor_copy(dst, ps[:])
                    else:
                        nc.scalar.mul(dst, ps[:], 1.0)
                # single HWDGE ring (SP): keeps DMA dispatch off ACT,
                # which is loaded with evictions
                nc.sync.dma_start(g[m * 128:(m + 1) * 128, cs:cs + w],
                                  strip[:])
                cs += w

        # slot ranges, by content, for transpose scheduling
        def slots_of_gbs(glo, ghi):
            return [s for s in range(NB)
                    if SLOT_CONTENT[s][0] == "gb"
                    and glo <= SLOT_CONTENT[s][1] < ghi]

        def slots_of_rows(mlo, mhi):
            return [s for s in range(NB)
                    if SLOT_CONTENT[s][0] == "row"
                    and mlo <= SLOT_CONTENT[s][1] < mhi]

        def transpose_slots(slots):
            slots = sorted(slots)
            i = 0
            while i < len(slots):
                j = i + 1
                while j < len(slots) and slots[j] == slots[i] + (j - i):
                    j += 1
                transposes(slots[i], slots[j - 1] + 1)
                i = j

        # ---- pipelined emission, staircase top-down ---------------------
        rec_chains(0, 1)        # set0: row15+gb120..127; set1: r13,14+gb104..119
        softmax_weights()
        rec_par(0)
        convert_set(0)
        transpose_slots(slots_of_rows(15, 16) + slots_of_gbs(120, 128))
        gemm_m(15)
        rec_par(1)
        convert_set(1)
        transpose_slots(slots_of_gbs(112, 120) + slots_of_rows(13, 15))
        gemm_m(14)
        rec_chains(2, 3)        # set2: rows 0..12+gb96..103; set3: gb64..95
        transpose_slots(slots_of_gbs(104, 112))
        gemm_m(13)
        rec_par(2)
        convert_set(2)
        transpose_slots(slots_of_gbs(96, 104) + slots_of_rows(8, 13))
        gemm_m(12)
        rec_par(3)
        convert_set(3)
        transpose_slots(slots_of_gbs(88, 96) + slots_of_rows(4, 8))
        gemm_m(11)
        transpose_slots(slots_of_gbs(80, 88))
        gemm_m(10)
        rec_chains(4, 5)        # set4: gb 32..63; set5: gb 0..31
        transpose_slots(slots_of_gbs(72, 80))
        gemm_m(9)
        transpose_slots(slots_of_gbs(64, 72))
        gemm_m(8)
        rec_par(4)
        convert_set(4)
        transpose_slots(slots_of_gbs(56, 64) + slots_of_rows(0, 4))
        gemm_m(7)
        transpose_slots(slots_of_gbs(48, 56))
        gemm_m(6)
        transpose_slots(slots_of_gbs(40, 48))
        gemm_m(5)
        rec_par(5)
        convert_set(5)
        transpose_slots(slots_of_gbs(32, 40))
        gemm_m(4)
        transpose_slots(slots_of_gbs(24, 32))
        gemm_m(3)
        transpose_slots(slots_of_gbs(16, 24))
        gemm_m(2)
        transpose_slots(slots_of_gbs(8, 16))
        gemm_m(1)
        transpose_slots(slots_of_gbs(0, 8))
        gemm_m(0)

    nc.compile()
    return nc


def _get_nc():
    if "nc" not in _CACHE:
        _CACHE["nc"] = _build_nc()
    return _CACHE["nc"]


def _make_in_maps(xs, logits):
    xs = np.ascontiguousarray(np.asarray(xs, dtype=np.float32).reshape(N_PTS))
    lg = np.ascontiguousarray(
        np.asarray(logits, dtype=np.float32).reshape(1, MAX_N + 1))
    xa = xs.reshape(128, 128)
    in_maps = []
    for c in range(N_CORES):
        # row tile m of core c is global row tile 8m+c
        rows = np.stack([xs[1024 * m + 128 * c:1024 * m + 128 * (c + 1)]
                         for m in range(N_ROW_BLOCKS)])
        in_maps.append({
            "xs_all": xa,
            "xs_rows": np.ascontiguousarray(rows),
            "logits": lg,
        })
    return in_maps


def run(xs, logits, trace=False, tmpdir=None):
    """Run the SPMD kernel; returns (full output, BassKernelResults)."""
    from concourse.bass_utils import run_bass_kernel_spmd

    nc = _get_nc()
    in_maps = _make_in_maps(xs, logits)
    res = run_bass_kernel_spmd(nc, in_maps, list(range(N_CORES)),
                               trace=trace, tmpdir=tmpdir)
    # assemble the upper staircase, then mirror the strict lower
    # triangle (device computes G[i,j] and G[j,i] identically, so the
    # mirror is bit-exact)
    out = np.zeros((N_PTS, N_PTS), np.float32)
    for c in range(N_CORES):
        gc = np.asarray(res.results[c]["g"], dtype=np.float32)
        for m in range(N_ROW_BLOCKS):
            r0 = 1024 * m + 128 * c
            out[r0:r0 + 128, 1024 * m:] = gc[128 * m:128 * (m + 1),
                                             1024 * m:]
    for m in range(1, N_ROW_BLOCKS):
        out[1024 * m:1024 * (m + 1), 0:1024 * m] = \
            out[0:1024 * m, 1024 * m:1024 * (m + 1)].T
    return out, res


def kernel(xs, logits):
    out, _ = run(xs, logits, trace=False)
    return out
